# revision 10
# baseline (speedup 1.0000x reference)
"""Trainium2 Bass kernel for nn_CapsuleLayer (dynamic routing, 3 iterations).

Math (reference):
    u_hat[b,c,u,s] = sum_i W[c,u,s,i] x[b,i,c]          (B=256,C=1152,U=10,S=16,I=8)
    3x routing:  c_ij = softmax_u(b_ij);  s_j = sum_c c_ij*u_hat;  v = squash(s_j)
                 b_ij += mean_b(u_hat . v)
    return v[..., None]

Sharding: everything heavy is sharded over K-chunks (9 of 72 per core, chunk
= 16 c x 8 i on 128 partitions) at FULL batch; the only collective is one
[256,160] fp32 AllReduce of the s-partials per routing iteration (3 total):
    s-partial[b,us] = sum_{own ci} x[ci,b] (c.W)[ci,us]   (PE, 18 bf16 matmuls)
      -> AllReduce(add) -> full s on every core -> squash locally (all 256 b)
    G[ci,us] = sum_b x[b,ci] v[b,us]  (PE, own chunks, v already local)
    a[c,u] = sum_{i,s} W.G via fused DVE prefix-scan + PE selection matmuls
    softmax + weff (W*c) stay core-local: only own chunks ever need c.
Every core ends with the identical full output (host reads core 0's copy).

Note: accumulating two PSUM regions of one tile with interleaved start/stop
groups corrupts the first region — keep each accumulation group's matmuls
consecutive (h-outer loop in s_pass).
"""

import numpy as np

import concourse.bass as bass
import concourse.bacc as bacc
import concourse.tile as tile
from concourse import mybir
from concourse import bass_utils

# ------------------------------------------- custom DVE op: prefix(W*G)


def _register_mulscan():
    import numpy as np
    from concourse import dve_ops
    from concourse.dve_spec import Spec, Src0, Src1, AluOp, scan, lower
    from concourse.dve_uop import DveOpSpec

    name = "CAPS_MULSCAN_ANT"
    for op in dve_ops.OPS:
        if op.name == name:
            return op
    spec = Spec(
        body=scan(AluOp.ADD, Src0 * Src1),
        reference=lambda in0, in1, s0, s1, imm2: np.cumsum(
            np.asarray(in0, np.float32).reshape(in0.shape[0], -1)
            * np.asarray(in1, np.float32).reshape(in1.shape[0], -1),
            axis=1,
        ),
    )
    row = dve_ops._CUSTOM_DVE_ROW_BASE + len(dve_ops.OPS)
    shas = {}
    for ver in ("v3", "v4"):
        s = DveOpSpec(name=name, opcode=row, uops=lower(spec, ver=ver), rd1_en=True)
        shas[ver] = s.sha(ver)
    op = dve_ops.DveOp(name, spec, subdim=False, uops_sha=shas)
    dve_ops.OPS.append(op)
    dve_ops.CUSTOM_DVE_SPECS[name] = spec
    dve_ops._SUB_OPCODE_FOR_NAME[name] = row
    return op


MULSCAN = _register_mulscan()


def _pin_act_tables():
    """Make natural_log_exp_and_others the unique candidate set for
    exp/ln/square so bacc's table-load pass never alternates sets."""
    import functools
    import concourse.bacc as _bacc
    import concourse.hw_specs as _hw
    if getattr(_bacc, "_caps_act_pinned", False):
        return
    orig = _hw.get_activation_tables

    @functools.cache
    def pinned(module_arch):
        tables = dict(orig(module_arch))
        keep = "natural_log_exp_and_others"
        assert keep in tables
        only = tables[keep]
        excl = {f for f in only}
        out = {}
        for name, funcs in tables.items():
            if name == keep:
                out[name] = funcs
            else:
                out[name] = funcs - excl
        return out

    _bacc.get_activation_tables = pinned
    _hw.get_activation_tables_orig = orig
    _bacc._caps_act_pinned = True


_pin_act_tables()

# ---------------------------------------------------------------- constants
B, I, C, U, S = 256, 8, 1152, 10, 16
NCORES = 8
BL = B // NCORES            # 32 batches per core
KT = C // 16                # 72 chunks of 128 partitions (16 c x 8 i)
KO = KT // NCORES           # 9 chunks owned per core
NUS = U * S                 # 160
EPS = 1e-8
NUM_ROUTING = 3
DVE_CHUNKS = 6              # weff chunks on DVE (rest on Pool)


# ---------------------------------------------------------------- device code
def build_nc(repeat=1, collectives=True, routing=NUM_ROUTING):
    nc = bacc.Bacc(
        "TRN2",
        target_bir_lowering=False,
        debug=False,
        num_devices=NCORES if collectives else 1,
    )
    bdt = mybir.dt.bfloat16
    f32 = mybir.dt.float32
    GRP = [list(range(NCORES))]

    w_d = nc.dram_tensor("w_own", [128, KO * NUS], bdt, kind="ExternalInput")
    w32_d = nc.dram_tensor("w32", [128, KO * NUS], f32, kind="ExternalInput")
    xts_d = nc.dram_tensor("x_ts", [128, KO * 2 * 128], bdt, kind="ExternalInput")
    xbt_d = nc.dram_tensor("x_bt", [128, 2 * KO * 128], bdt, kind="ExternalInput")
    sel_d = nc.dram_tensor("sel", [128, 16], f32, kind="ExternalInput")
    seln_d = nc.dram_tensor("seln", [128, 16], f32, kind="ExternalInput")
    out_d = nc.dram_tensor("v_out", [B, NUS], f32, kind="ExternalOutput")

    aspace = "Shared" if collectives else "Local"
    rs_ins = [
        nc.dram_tensor(f"rs_in{p}", [B, NUS], f32, kind="Internal")
        for p in range(3)
    ]
    # AllReduce -> full s on every core each iteration: squash runs on the
    # full batch locally (no v gather), and every core emits the full output.
    ar_outs = [
        nc.dram_tensor(f"ar_out{p}", [B, NUS], f32, kind="Internal",
                       addr_space=aspace)
        for p in range(3)
    ]

    with tile.TileContext(nc) as tc:
        with (
            tc.tile_pool(name="singles", bufs=1) as singles,
            tc.tile_pool(name="weff_p", bufs=2) as weff_p,
            tc.tile_pool(name="prod_p", bufs=2) as prod_p,
            tc.tile_pool(name="small", bufs=2) as small,
            tc.tile_pool(name="bsoft", bufs=2) as bsoft,
            tc.tile_pool(name="ps_s", bufs=2, space="PSUM") as ps_s,
            tc.tile_pool(name="ps_g", bufs=1, space="PSUM") as ps_g,
            tc.tile_pool(name="ps_b", bufs=1, space="PSUM") as ps_b,
        ):
            # ---------------- persistent SBUF loads
            w_own = singles.tile([128, KO, U, S], bdt)
            nc.sync.dma_start(out=w_own[:], in_=w_d[:])
            w32 = singles.tile([128, KO, U, S], f32)
            nc.sync.dma_start(out=w32[:], in_=w32_d[:])
            x_ts = singles.tile([128, KO, 2, 128], bdt)
            nc.sync.dma_start(out=x_ts[:], in_=xts_d[:])
            x_bt = singles.tile([128, 2, KO, 128], bdt)
            nc.sync.dma_start(out=x_bt[:], in_=xbt_d[:])
            sel = singles.tile([128, 16], f32)
            nc.sync.dma_start(out=sel[:], in_=sel_d[:])
            seln = singles.tile([128, 16], f32)
            nc.sync.dma_start(out=seln[:], in_=seln_d[:])
            b_own = singles.tile([16, KO * U], f32)
            eps_sb = singles.tile([128, 1], f32)
            nc.vector.memset(eps_sb[:], EPS)
            one_sb = singles.tile([128, 1], f32)
            nc.vector.memset(one_sb[:], 1.0)
            lna_sb = singles.tile([128, 1], f32)
            nc.vector.memset(lna_sb[:], float(np.log(1.0 / U)))

            def s_pass(c_exp, it):
                """s-partial over own 9 chunks at full batch -> RS -> own s."""
                if c_exp is None:
                    flat = w_own[:].rearrange("p k u s -> p (k u s)")
                else:
                    weff = weff_p.tile([128, KO, U, S], bdt, tag="weff")
                    nc.vector.tensor_mul(
                        weff[:, :DVE_CHUNKS],
                        w_own[:, :DVE_CHUNKS],
                        c_exp[:, :DVE_CHUNKS, :, None].broadcast_to(
                            [128, DVE_CHUNKS, U, S]
                        ),
                    )
                    nc.gpsimd.tensor_mul(
                        weff[:, DVE_CHUNKS:],
                        w_own[:, DVE_CHUNKS:],
                        c_exp[:, DVE_CHUNKS:, :, None].broadcast_to(
                            [128, KO - DVE_CHUNKS, U, S]
                        ),
                    )
                    flat = weff[:].rearrange("p k u s -> p (k u s)")
                sp = ps_s.tile([128, 2, NUS], f32, tag="sp")
                for h in range(2):
                    for ko in range(KO):
                        nc.tensor.matmul(
                            out=sp[:, h, :],
                            lhsT=x_ts[:, ko, h, :],
                            rhs=flat[:, ko * NUS:(ko + 1) * NUS],
                            start=ko == 0,
                            stop=ko == KO - 1,
                        )
                    # drain each half as soon as its group stops so h0's
                    # psum->sbuf copy + DRAM DMA overlap h1's matmuls
                    s_sb = small.tile([128, NUS], f32, tag=f"s_sb{h}")
                    nc.scalar.copy(out=s_sb[:], in_=sp[:, h, :])
                    nc.sync.dma_start(
                        out=rs_ins[it][128 * h:128 * (h + 1), :],
                        in_=s_sb[:],
                    )
                if collectives:
                    nc.gpsimd.collective_compute(
                        "AllReduce",
                        mybir.AluOpType.add,
                        replica_groups=GRP,
                        ins=[rs_ins[it][:].opt()],
                        outs=[ar_outs[it][:].opt()],
                    )
                else:
                    nc.sync.dma_start(out=ar_outs[it][:], in_=rs_ins[it][:])
                s_all = small.tile([128, 2, U, S], f32, tag="s_all")
                for h in range(2):
                    nc.sync.dma_start(
                        out=s_all[:, h, :, :],
                        in_=ar_outs[it][128 * h:128 * (h + 1), :],
                    )
                return s_all

            def squash(s_in, alpha, out_dt, tag, P, G):
                """v = squash(alpha*s) = s*exp(.5*ln(a^2 t+eps)-ln(a^2 t+1)
                +ln a);  t = sum_s s^2.  s_in: [P, G, U, S]."""
                s2 = small.tile([P, G, U, S], f32, tag=tag + "s2")
                nc.scalar.activation(
                    out=s2[:], in_=s_in,
                    func=mybir.ActivationFunctionType.Square,
                )
                t = small.tile([P, G, U], f32, tag=tag + "t")
                nc.vector.reduce_sum(out=t[:], in_=s2[:], axis=mybir.AxisListType.X)
                lnt = small.tile([P, G, U], f32, tag=tag + "lnt")
                nc.scalar.activation(
                    out=lnt[:], in_=t[:],
                    func=mybir.ActivationFunctionType.Ln,
                    scale=float(alpha * alpha), bias=eps_sb[:P, :],
                )
                ln1 = small.tile([P, G, U], f32, tag=tag + "ln1")
                nc.scalar.activation(
                    out=ln1[:], in_=t[:],
                    func=mybir.ActivationFunctionType.Ln,
                    scale=float(alpha * alpha), bias=one_sb[:P, :],
                )
                h = small.tile([P, G, U], f32, tag=tag + "h")
                nc.vector.scalar_tensor_tensor(
                    out=h[:], in0=lnt[:], scalar=0.5, in1=ln1[:],
                    op0=mybir.AluOpType.mult, op1=mybir.AluOpType.subtract,
                )
                m = small.tile([P, G, U], f32, tag=tag + "m")
                if alpha == 1.0:
                    nc.scalar.activation(
                        out=m[:], in_=h[:],
                        func=mybir.ActivationFunctionType.Exp,
                    )
                else:
                    nc.scalar.activation(
                        out=m[:], in_=h[:],
                        func=mybir.ActivationFunctionType.Exp,
                        bias=lna_sb[:P, :],
                    )
                v = small.tile([P, G, U, S], out_dt, tag=tag)
                nc.vector.tensor_mul(
                    v[:], s_in, m[:, :, :, None].broadcast_to([P, G, U, S])
                )
                return v

            def a_pass(v_all, it):
                """G over own chunks at full batch (rhs = locally squashed
                v_all), fused W.G prefix-scan, selection matmuls -> b_ps."""
                g_ps = ps_g.tile([128, 3, 512], f32, tag="g")
                for ko in range(KO):
                    for h in range(2):
                        nc.tensor.matmul(
                            out=g_ps[:, ko // 3, (ko % 3) * NUS:(ko % 3 + 1) * NUS],
                            lhsT=x_bt[:, h, ko, :],
                            rhs=v_all[:, h, :, :].rearrange("p u s -> p (u s)"),
                            start=h == 0,
                            stop=h == 1,
                        )
                nseg = KO * U
                pref = prod_p.tile([128, S * (nseg + 1)], f32, tag="pref")
                nc.scalar.mul(out=pref[:, 0:1], in_=sel[:, 0:1], mul=0.0)
                nc.vector._custom_dve(
                    MULSCAN,
                    out=pref[:, 1:1 + KO * NUS],
                    in0=w32[:].rearrange("p k u s -> p (k u s)"),
                    in1=g_ps[:, :, :3 * NUS],
                )
                ends = pref[:, S:S + nseg * S].rearrange(
                    "p (n s) -> p n s", s=S
                )[:, :, 0]
                prevs = pref[:, 0:nseg * S].rearrange(
                    "p (n s) -> p n s", s=S
                )[:, :, 0]
                b_ps = ps_b.tile([16, nseg], f32, tag="b_ps")
                nc.tensor.matmul(
                    out=b_ps[:], lhsT=sel[:], rhs=ends, start=True, stop=False
                )
                nc.tensor.matmul(
                    out=b_ps[:], lhsT=seln[:], rhs=prevs, start=False, stop=True
                )
                return b_ps

            def logits_softmax(b_ps, it):
                """b += a (own chunks); softmax over u; local partition-
                broadcast expand to c_exp [128, KO, U]."""
                if it == 0:
                    nc.scalar.copy(out=b_own[:], in_=b_ps[:])
                else:
                    nc.vector.tensor_add(b_own[:], b_own[:], b_ps[:])
                b3 = b_own[:].rearrange("c (k u) -> c k u", u=U)
                e = bsoft.tile([16, KO, U], f32, tag="e")
                nc.scalar.activation(
                    out=e[:], in_=b3, func=mybir.ActivationFunctionType.Exp
                )
                se = bsoft.tile([16, KO], f32, tag="se")
                nc.vector.reduce_sum(out=se[:], in_=e[:], axis=mybir.AxisListType.X)
                re = bsoft.tile([16, KO], f32, tag="re")
                nc.vector.reciprocal(re[:], se[:])
                c_own = bsoft.tile([16, KO, U], bdt, tag="c_own")
                nc.vector.tensor_mul(
                    c_own[:], e[:], re[:, :, None].broadcast_to([16, KO, U])
                )
                c_exp = bsoft.tile([128, KO, U], bdt, tag="c_exp")
                nc.sync.dma_start(
                    out=c_exp[:],
                    in_=c_own[:, None, :, :].broadcast_to([16, 8, KO, U]),
                )
                return c_exp

            # ------------------------------------------------ routing loop
            for _rep in range(repeat):
                c_exp = None
                for it in range(routing):
                    alpha = 1.0 / U if it == 0 else 1.0
                    s_x = s_pass(c_exp, it)
                    if it == routing - 1:
                        v = squash(s_x[:], alpha, mybir.dt.float32,
                                   "v32", 128, 2)
                        break
                    v_all = squash(s_x[:], alpha, bdt, "va", 128, 2)[:]
                    b_ps = a_pass(v_all, it)
                    c_exp = logits_softmax(b_ps, it)

                for h in range(2):
                    nc.sync.dma_start(
                        out=out_d[128 * h:128 * (h + 1), :],
                        in_=v[:, h, :, :].rearrange("p u s -> p (u s)"),
                    )

    nc.compile()
    return nc


# ---------------------------------------------------------------- host prep
def prep_inputs(x, weight):
    """Full inputs -> per-core in_maps with kernel-ready layouts."""
    import ml_dtypes
    bf = ml_dtypes.bfloat16
    x = np.asarray(x, dtype=np.float32)
    weight = np.asarray(weight, dtype=np.float32)

    # W: [C,U,S,I] -> [128, KT, U, S] with p = (c%16)*8 + i
    w = (
        weight.reshape(KT, 16, U, S, I)
        .transpose(1, 4, 0, 2, 3)
        .reshape(128, KT, U * S)
    )
    sel = np.zeros((128, 16), np.float32)
    sel[np.arange(128), np.arange(128) // 8] = 1.0 / B
    xb_all = x.transpose(0, 2, 1).reshape(B, KT, 16 * I)     # [b, k, q]
    # x in (c,i)-partition layout at full batch: [128, KT, B]
    x_full = (
        x.transpose(2, 1, 0).reshape(KT, 16, I, B).reshape(KT, 128, B)
        .transpose(1, 0, 2)                                   # [128, KT, B]
    )

    in_maps = []
    for m in range(NCORES):
        ko = slice(m * KO, (m + 1) * KO)
        w_own = np.ascontiguousarray(w[:, ko, :], dtype=bf).reshape(128, -1)
        w32 = np.ascontiguousarray(w[:, ko, :], dtype=np.float32).reshape(
            128, -1
        )
        x_ts = np.ascontiguousarray(
            x_full[:, ko, :].reshape(128, KO, 2, 128), dtype=bf
        ).reshape(128, -1)
        x_bt = (
            xb_all[:, ko, :]                                  # [256, KO, 128]
            .reshape(2, 128, KO, 128)
            .transpose(1, 0, 2, 3)                            # [128, 2, KO, 128]
            .reshape(128, -1)
        )
        in_maps.append({
            "w_own": w_own,
            "w32": w32,
            "x_ts": x_ts,
            "x_bt": np.ascontiguousarray(x_bt, dtype=bf),
            "sel": sel,
            "seln": -sel,
        })
    return in_maps


def assemble_output(results):
    return np.ascontiguousarray(
        results[0]["v_out"].astype(np.float32).reshape(B, U, S, 1)
    )


_NC_CACHE = {}


def _get_nc():
    if "nc" not in _NC_CACHE:
        _NC_CACHE["nc"] = build_nc()
    return _NC_CACHE["nc"]


def kernel(x, weight):
    nc = _get_nc()
    in_maps = prep_inputs(x, weight)
    res = bass_utils.run_bass_kernel_spmd(
        nc, in_maps, core_ids=list(range(NCORES))
    )
    return assemble_output(res.results)
